# revision 19
# baseline (speedup 1.0000x reference)
"""Masked-gather L1 loss on 8 Trainium2 NeuronCores.

Strategy (data-parallel over batch, 4 batches per core, chunked over HW):
  - Each batch's pred[b] ([128 c, 25600 hw] f32, 13.1 MB) streams into SBUF
    in column chunks (4x6400 for batch slots 0-2; descending
    8192/8192/5120/2048/2048 for the last slot so the final gather is tiny).
  - Host buckets each batch's 1024 indices by chunk, pads each bucket to a
    shared per-(slot,chunk) capacity (multiple of 16, max over cores), and
    permutes target/mask into the same bucketed order (pad: idx=-1, mask=0).
  - GPSIMD ap_gather pulls each chunk's bucketed columns out of SBUF while
    later chunks are still streaming; a tiny warmup gather at t=0 hoists the
    GPSIMD library load off the critical path.
  - Per chunk: DVE subtract, ACT abs, PE ones^T matmul -> per-k column sums
    in PSUM, DVE mask-multiply + reduce -> one scalar per chunk.
  - Small DMAs (idx/target/mask/out) ride the scalar HWDGE ring so the sync
    ring carries only the back-to-back pred chunk streams.
  - Each core returns per-chunk partial sums + per-batch mask sums; host
    combines all 8 cores and applies total / (mask_sum * C + eps).
"""

import sys

sys.path.insert(0, "/opt/trn_rl_repo")

import ml_dtypes
import numpy as np

B, C, H, W = 32, 128, 160, 160
K = 1024
HW = H * W
N_CORES = 8
BPC = B // N_CORES  # batches per core
EPS = 1e-5

SLOT_WIDTHS = [
    [6400, 6400, 6400, 6400],
    [6400, 6400, 6400, 6400],
    [6400, 6400, 6400, 6400],
    [8704, 8704, 5120, 1536, 1536],
]
NCHUNKS = sum(len(w) for w in SLOT_WIDTHS)

_CACHE = {}


def _roundup16(n):
    return max(16, (n + 15) // 16 * 16)


def _bucketize(indices):
    """Bucket each batch's indices by chunk; capacities shared across cores."""
    sel = {}
    counts = {}
    for core in range(N_CORES):
        for i in range(BPC):
            idxb = indices[core * BPC + i]
            base = 0
            for j, w in enumerate(SLOT_WIDTHS[i]):
                m = (idxb >= base) & (idxb < base + w)
                ks = np.nonzero(m)[0]
                sel[(core, i, j)] = (ks, (idxb[ks] - base).astype(np.int16))
                counts[(core, i, j)] = len(ks)
                base += w
    caps = tuple(
        tuple(
            _roundup16(max(counts[(core, i, j)] for core in range(N_CORES)))
            for j in range(len(SLOT_WIDTHS[i]))
        )
        for i in range(BPC)
    )
    return caps, sel


def _build(caps):
    from contextlib import ExitStack

    from concourse import bacc, mybir, tile

    f32 = mybir.dt.float32
    bf16 = mybir.dt.bfloat16
    i16 = mybir.dt.int16

    kp = [sum(c) for c in caps]  # padded K per batch slot
    totk = sum(kp)
    capmax = max(max(c) for c in caps)
    wmax = max(max(w) for w in SLOT_WIDTHS)

    nc = bacc.Bacc(
        "TRN2",
        target_bir_lowering=False,
        debug=False,
        num_devices=N_CORES,
        dynamic_dma_scratch_size=4096,
    )

    pred_d = nc.dram_tensor("pred", [BPC, C, HW], f32, kind="ExternalInput")
    tgt_d = nc.dram_tensor("tgt", [C, totk], bf16, kind="ExternalInput")
    idx_d = nc.dram_tensor("idx", [C, totk // 16], i16, kind="ExternalInput")
    mask_d = nc.dram_tensor("mask", [1, totk], f32, kind="ExternalInput")
    out_d = nc.dram_tensor("out", [1, NCHUNKS + BPC], f32, kind="ExternalOutput")

    with tile.TileContext(nc) as tc, ExitStack() as ctx:
        pred_pool = ctx.enter_context(tc.tile_pool(name="pred", bufs=4))
        gt_pool = ctx.enter_context(tc.tile_pool(name="gt", bufs=4))
        singles = ctx.enter_context(tc.tile_pool(name="singles", bufs=1))
        psum = ctx.enter_context(tc.tile_pool(name="psum", bufs=2, space="PSUM"))

        # Warmup gather: first GPSIMD op, so the ap_gather ucode library
        # loads at t=0, overlapped with the first pred chunk's DMA.
        warm_in = singles.tile([16, 16], f32)
        warm_idx = singles.tile([16, 1], i16)
        warm_out = singles.tile([16, 16], f32)
        nc.vector.memset(warm_in[:], 0.0)
        nc.vector.memset(warm_idx[:], 0)
        nc.gpsimd.ap_gather(
            warm_out[:], warm_in[:], warm_idx[:],
            channels=16, num_elems=16, d=1, num_idxs=16,
        )

        idx_t = singles.tile([C, totk // 16], i16)
        nc.scalar.dma_start(idx_t[:], idx_d.ap()[:])
        tgt16_t = singles.tile([C, totk], bf16)
        nc.scalar.dma_start(tgt16_t[:], tgt_d.ap()[:])
        tgt_t = singles.tile([C, totk], f32)
        nc.scalar.activation(
            tgt_t[:], tgt16_t[:], mybir.ActivationFunctionType.Copy
        )
        msk_t = singles.tile([1, totk], f32)
        nc.scalar.dma_start(msk_t[:], mask_d.ap()[:])

        ones_t = singles.tile([C, 1], f32)
        nc.vector.memset(ones_t[:], 1.0)
        acc_t = singles.tile([1, NCHUNKS + BPC], f32)

        # Per-batch mask sums early (off the tail; pads carry mask=0).
        koff = 0
        for i in range(BPC):
            nc.vector.tensor_reduce(
                acc_t[:, NCHUNKS + i : NCHUNKS + i + 1],
                msk_t[:, koff : koff + kp[i]],
                axis=mybir.AxisListType.X,
                op=mybir.AluOpType.add,
            )
            koff += kp[i]

        col = 0
        koff = 0
        for i in range(BPC):
            base = 0
            for j, w in enumerate(SLOT_WIDTHS[i]):
                cap = caps[i][j]
                pt = pred_pool.tile([C, wmax], f32)
                nc.sync.dma_start(pt[:, :w], pred_d.ap()[i, :, base : base + w])

                gt = gt_pool.tile([C, capmax], f32)
                nc.gpsimd.ap_gather(
                    gt[:, :cap],
                    pt[:, :w],
                    idx_t[:, koff // 16 : (koff + cap) // 16],
                    channels=C,
                    num_elems=w,
                    d=1,
                    num_idxs=cap,
                )
                nc.vector.tensor_tensor(
                    gt[:, :cap], gt[:, :cap], tgt_t[:, koff : koff + cap],
                    op=mybir.AluOpType.subtract,
                )
                nc.scalar.activation(
                    gt[:, :cap], gt[:, :cap], mybir.ActivationFunctionType.Abs
                )
                ps = psum.tile([1, cap], f32)
                for s in range(0, cap, 512):
                    e = min(cap, s + 512)
                    nc.tensor.matmul(ps[:, s:e], ones_t[:], gt[:, s:e])
                tm = gt_pool.tile([1, capmax], f32)
                nc.vector.scalar_tensor_tensor(
                    tm[:, :cap],
                    ps[:],
                    1.0,
                    msk_t[:, koff : koff + cap],
                    op0=mybir.AluOpType.mult,
                    op1=mybir.AluOpType.mult,
                    accum_out=acc_t[:, col : col + 1],
                )
                col += 1
                koff += cap
                base += w

        nc.scalar.dma_start(out_d.ap()[:], acc_t[:])

    nc.compile()
    return nc


def _get_nc(caps):
    key = ("nc", caps)
    if key not in _CACHE:
        _CACHE[key] = _build(caps)
    return _CACHE[key]


def make_in_maps(pred, target, indices, mask, caps, sel):
    pred = np.ascontiguousarray(np.asarray(pred), dtype=np.float32)
    target = np.ascontiguousarray(np.asarray(target), dtype=np.float32)
    mask = np.ascontiguousarray(np.asarray(mask), dtype=np.float32)

    kp = [sum(c) for c in caps]
    totk = sum(kp)
    predf = pred.reshape(B, C, HW)

    in_maps = []
    for core in range(N_CORES):
        tgt_flat = np.zeros((C, totk), dtype=ml_dtypes.bfloat16)
        msk_flat = np.zeros((1, totk), dtype=np.float32)
        idx_flat = np.full((C, totk // 16), -1, dtype=np.int16)
        koff = 0
        for i in range(BPC):
            b = core * BPC + i
            for j in range(len(SLOT_WIDTHS[i])):
                cap = caps[i][j]
                ks, loc = sel[(core, i, j)]
                n = len(ks)
                tgt_flat[:, koff : koff + n] = target[b][:, ks]
                msk_flat[0, koff : koff + n] = mask[b][ks]
                arr = np.full(cap, -1, dtype=np.int16)
                arr[:n] = loc
                wrapped = arr.reshape(cap // 16, 16).T  # [16, cap//16]
                idx_flat[:, koff // 16 : (koff + cap) // 16] = np.tile(
                    wrapped, (C // 16, 1)
                )
                koff += cap
        in_maps.append(
            {
                "pred": np.ascontiguousarray(predf[core * BPC : (core + 1) * BPC]),
                "tgt": tgt_flat,
                "idx": idx_flat,
                "mask": msk_flat,
            }
        )
    return in_maps


def run(pred, target, indices, mask, trace=False, **rk_kwargs):
    from concourse.bass_utils import run_bass_kernel_spmd

    indices = np.asarray(indices).astype(np.int64)
    caps, sel = _bucketize(indices)
    nc = _get_nc(caps)
    in_maps = make_in_maps(pred, target, indices, mask, caps, sel)
    res = run_bass_kernel_spmd(
        nc, in_maps, list(range(N_CORES)), trace=trace, **rk_kwargs
    )
    parts = np.stack([r["out"][0] for r in res.results])  # [8, NCHUNKS+BPC]
    total = float(parts[:, :NCHUNKS].sum())
    mask_sum = float(parts[:, NCHUNKS:].sum())
    out = np.float32(total / (mask_sum * C + EPS))
    return out, res


def kernel(pred, target, indices, mask):
    out, _ = run(pred, target, indices, mask)
    return out


# revision 22
# speedup vs baseline: 1.1828x; 1.1828x over previous
"""Masked-gather L1 loss on 8 Trainium2 NeuronCores.

Strategy (data-parallel over batch, 4 batches per core, chunked over HW):
  - Each batch's pred[b] ([128 c, 25600 hw] f32, 13.1 MB) streams into SBUF
    in column chunks (4x6400 for batch slots 0-2; descending
    8704/8704/5120/1536/1536 for the last slot so the final gather is tiny).
  - Host buckets each batch's 1024 indices by chunk, pads each bucket to a
    shared per-(slot,chunk) capacity (multiple of 16, max over cores), and
    permutes target/mask into the same bucketed order (pad: idx=-1, mask=0).
  - GPSIMD ap_gather pulls each chunk's bucketed columns out of SBUF while
    later chunks are still streaming; a tiny warmup gather at t=0 hoists the
    GPSIMD library load off the critical path.
  - Per chunk: DVE subtract, ACT abs, PE ones^T matmul -> per-k column sums
    in PSUM, DVE mask-multiply + reduce -> one scalar per chunk.
  - Small DMAs (idx/target/mask/out) ride the scalar HWDGE ring so the sync
    ring carries only the back-to-back pred chunk streams.
  - Each core returns per-chunk partial sums + per-batch mask sums; host
    combines all 8 cores and applies total / (mask_sum * C + eps).
"""

import sys

sys.path.insert(0, "/opt/trn_rl_repo")

import ml_dtypes
import numpy as np

B, C, H, W = 32, 128, 160, 160
K = 1024
HW = H * W
N_CORES = 8
BPC = B // N_CORES  # batches per core
EPS = 1e-5

SLOT_WIDTHS = [
    [6400, 6400, 6400, 6400],
    [6400, 6400, 6400, 6400],
    [6400, 6400, 6400, 6400],
    [8704, 8704, 5120, 1536, 1536],
]
NCHUNKS = sum(len(w) for w in SLOT_WIDTHS)

_CACHE = {}


def _roundup16(n):
    return max(16, (n + 15) // 16 * 16)


def _bucketize(indices):
    """Bucket each batch's indices by chunk; capacities shared across cores."""
    sel = {}
    counts = {}
    for core in range(N_CORES):
        for i in range(BPC):
            idxb = indices[core * BPC + i]
            base = 0
            for j, w in enumerate(SLOT_WIDTHS[i]):
                m = (idxb >= base) & (idxb < base + w)
                ks = np.nonzero(m)[0]
                sel[(core, i, j)] = (ks, (idxb[ks] - base).astype(np.int16))
                counts[(core, i, j)] = len(ks)
                base += w
    caps = tuple(
        tuple(
            _roundup16(max(counts[(core, i, j)] for core in range(N_CORES)))
            for j in range(len(SLOT_WIDTHS[i]))
        )
        for i in range(BPC)
    )
    return caps, sel


def _build(caps):
    from contextlib import ExitStack

    from concourse import bacc, mybir, tile

    f32 = mybir.dt.float32
    bf16 = mybir.dt.bfloat16
    i16 = mybir.dt.int16

    kp = [sum(c) for c in caps]  # padded K per batch slot
    totk = sum(kp)
    capmax = max(max(c) for c in caps)
    wmax = max(max(w) for w in SLOT_WIDTHS)

    nc = bacc.Bacc(
        "TRN2",
        target_bir_lowering=False,
        debug=False,
        num_devices=N_CORES,
        dynamic_dma_scratch_size=4096,
    )

    pred_d = nc.dram_tensor("pred", [BPC, C, HW], f32, kind="ExternalInput")
    tgt_d = nc.dram_tensor("tgt", [C, totk], bf16, kind="ExternalInput")
    idx_d = nc.dram_tensor("idx", [C, totk // 16], i16, kind="ExternalInput")
    mask_d = nc.dram_tensor("mask", [1, totk], f32, kind="ExternalInput")
    out_d = nc.dram_tensor("out", [1, NCHUNKS + BPC], f32, kind="ExternalOutput")

    with tile.TileContext(nc) as tc, ExitStack() as ctx:
        pred_pool = ctx.enter_context(tc.tile_pool(name="pred", bufs=4))
        gt_pool = ctx.enter_context(tc.tile_pool(name="gt", bufs=4))
        singles = ctx.enter_context(tc.tile_pool(name="singles", bufs=1))
        psum = ctx.enter_context(tc.tile_pool(name="psum", bufs=2, space="PSUM"))

        # Warmup gather: first GPSIMD op, so the ap_gather ucode library
        # loads at t=0, overlapped with the first pred chunk's DMA.
        warm_in = singles.tile([16, 16], f32)
        warm_idx = singles.tile([16, 1], i16)
        warm_out = singles.tile([16, 16], f32)
        nc.gpsimd.memset(warm_in[:], 0.0)
        nc.gpsimd.memset(warm_idx[:], 0)
        nc.gpsimd.ap_gather(
            warm_out[:], warm_in[:], warm_idx[:],
            channels=16, num_elems=16, d=1, num_idxs=16,
        )

        idx_t = singles.tile([C, totk // 16], i16)
        nc.scalar.dma_start(idx_t[:], idx_d.ap()[:])
        tgt16_t = singles.tile([C, totk], bf16)
        nc.scalar.dma_start(tgt16_t[:], tgt_d.ap()[:])
        tgt_t = singles.tile([C, totk], f32)
        nc.scalar.activation(
            tgt_t[:], tgt16_t[:], mybir.ActivationFunctionType.Copy
        )
        msk_t = singles.tile([1, totk], f32)
        nc.scalar.dma_start(msk_t[:], mask_d.ap()[:])

        ones_t = singles.tile([C, 1], f32)
        nc.vector.memset(ones_t[:], 1.0)
        acc_t = singles.tile([1, NCHUNKS + BPC], f32)

        # Per-batch mask sums early (off the tail; pads carry mask=0).
        koff = 0
        for i in range(BPC):
            nc.vector.tensor_reduce(
                acc_t[:, NCHUNKS + i : NCHUNKS + i + 1],
                msk_t[:, koff : koff + kp[i]],
                axis=mybir.AxisListType.X,
                op=mybir.AluOpType.add,
            )
            koff += kp[i]

        col = 0
        koff = 0
        for i in range(BPC):
            base = 0
            for j, w in enumerate(SLOT_WIDTHS[i]):
                cap = caps[i][j]
                pt = pred_pool.tile([C, wmax], f32)
                nc.sync.dma_start(pt[:, :w], pred_d.ap()[i, :, base : base + w])

                gt = gt_pool.tile([C, capmax], f32)
                nc.gpsimd.ap_gather(
                    gt[:, :cap],
                    pt[:, :w],
                    idx_t[:, koff // 16 : (koff + cap) // 16],
                    channels=C,
                    num_elems=w,
                    d=1,
                    num_idxs=cap,
                )
                nc.vector.tensor_tensor(
                    gt[:, :cap], gt[:, :cap], tgt_t[:, koff : koff + cap],
                    op=mybir.AluOpType.subtract,
                )
                nc.scalar.activation(
                    gt[:, :cap], gt[:, :cap], mybir.ActivationFunctionType.Abs
                )
                ps = psum.tile([1, cap], f32)
                for s in range(0, cap, 512):
                    e = min(cap, s + 512)
                    nc.tensor.matmul(ps[:, s:e], ones_t[:], gt[:, s:e])
                tm = gt_pool.tile([1, capmax], f32)
                nc.vector.scalar_tensor_tensor(
                    tm[:, :cap],
                    ps[:],
                    1.0,
                    msk_t[:, koff : koff + cap],
                    op0=mybir.AluOpType.mult,
                    op1=mybir.AluOpType.mult,
                    accum_out=acc_t[:, col : col + 1],
                )
                col += 1
                koff += cap
                base += w

        nc.scalar.dma_start(out_d.ap()[:], acc_t[:])

    nc.compile()
    return nc


def _get_nc(caps):
    key = ("nc", caps)
    if key not in _CACHE:
        _CACHE[key] = _build(caps)
    return _CACHE[key]


def make_in_maps(pred, target, indices, mask, caps, sel):
    pred = np.ascontiguousarray(np.asarray(pred), dtype=np.float32)
    target = np.ascontiguousarray(np.asarray(target), dtype=np.float32)
    mask = np.ascontiguousarray(np.asarray(mask), dtype=np.float32)

    kp = [sum(c) for c in caps]
    totk = sum(kp)
    predf = pred.reshape(B, C, HW)

    in_maps = []
    for core in range(N_CORES):
        tgt_flat = np.zeros((C, totk), dtype=ml_dtypes.bfloat16)
        msk_flat = np.zeros((1, totk), dtype=np.float32)
        idx_flat = np.full((C, totk // 16), -1, dtype=np.int16)
        koff = 0
        for i in range(BPC):
            b = core * BPC + i
            for j in range(len(SLOT_WIDTHS[i])):
                cap = caps[i][j]
                ks, loc = sel[(core, i, j)]
                n = len(ks)
                tgt_flat[:, koff : koff + n] = target[b][:, ks]
                msk_flat[0, koff : koff + n] = mask[b][ks]
                arr = np.full(cap, -1, dtype=np.int16)
                arr[:n] = loc
                wrapped = arr.reshape(cap // 16, 16).T  # [16, cap//16]
                idx_flat[:, koff // 16 : (koff + cap) // 16] = np.tile(
                    wrapped, (C // 16, 1)
                )
                koff += cap
        in_maps.append(
            {
                "pred": np.ascontiguousarray(predf[core * BPC : (core + 1) * BPC]),
                "tgt": tgt_flat,
                "idx": idx_flat,
                "mask": msk_flat,
            }
        )
    return in_maps


def run(pred, target, indices, mask, trace=False, **rk_kwargs):
    from concourse.bass_utils import run_bass_kernel_spmd

    indices = np.asarray(indices).astype(np.int64)
    caps, sel = _bucketize(indices)
    nc = _get_nc(caps)
    in_maps = make_in_maps(pred, target, indices, mask, caps, sel)
    res = run_bass_kernel_spmd(
        nc, in_maps, list(range(N_CORES)), trace=trace, **rk_kwargs
    )
    parts = np.stack([r["out"][0] for r in res.results])  # [8, NCHUNKS+BPC]
    total = float(parts[:, :NCHUNKS].sum())
    mask_sum = float(parts[:, NCHUNKS:].sum())
    out = np.float32(total / (mask_sum * C + EPS))
    return out, res


def kernel(pred, target, indices, mask):
    out, _ = run(pred, target, indices, mask)
    return out
